# revision 8
# baseline (speedup 1.0000x reference)
"""Trainium2 Bass kernel for GQA attention (nn_Attention_43181601194655).

Full module: hidden [B,S,HID] -> Wq/Wk/Wv proj -> RoPE -> causal GQA attention
-> Wo proj. Sharded tensor-parallel over heads across 8 NeuronCores:
core c owns q-heads [4c..4c+4) and kv-head c (Wq/Wk/Wv column slices, Wo row
slice). Each core computes a full-shape partial output; the host sums the 8
partials (the row-parallel Wo reduction).

Layout strategy (everything contracts on SBUF partitions):
- hidden^T (bf16, host-transposed) streams in as [128, C, s] tiles.
- Q^T/K^T produced directly by projection matmuls as [d, s] (d on partitions),
  RoPE applied with partition-shifted DVE multiplies.
- scores computed transposed S^T[k,q] = K^T.T-contracted; exp on ScalarE reads
  PSUM with fused softmax scale; causal handled by loop structure + one
  [128,128] additive mask on diagonal blocks.
- P^T[k,q] bf16 feeds PV as lhsT giving attn[q,d] with an appended ones column
  for the softmax denominator; per-partition reciprocal normalizes.
- attn pairs are PE-transposed to attn^T[hd, s] for the Wo matmul.
"""

import sys

if "/opt/trn_rl_repo" not in sys.path:
    sys.path.insert(0, "/opt/trn_rl_repo")

import numpy as np
import ml_dtypes

import concourse.bass as bass
from concourse import bacc
import concourse.mybir as mybir
from concourse.tile import TileContext
from concourse.masks import make_identity

BF16 = mybir.dt.bfloat16
F32 = mybir.dt.float32

B, S, HID = 2, 2048, 2048
H, HKV, D = 32, 8, 64
NCORES = 8
HQ = H // NCORES          # q heads per core (4)
HD = HQ * D               # 256: per-core attn feature dim
SCALE = D ** -0.5
SSUP = 512                # q supertile width
NEG = -1e9


def build_nc(b_sz=B, s_sz=S, hid=HID):
    """Build the per-core Bass program. Parameterized for small-sim testing."""
    C = hid // 128            # contraction chunks
    n_st = s_sz // 128        # 128-tiles along s
    n_sup = max(1, s_sz // SSUP)
    sup = min(SSUP, s_sz)
    n_qt = sup // 128         # q-tiles per supertile

    nc = bacc.Bacc()
    hsT = nc.dram_tensor("hsT", [hid, b_sz * s_sz], BF16, kind="ExternalInput")
    wq = nc.dram_tensor("wq", [hid, HQ * D], BF16, kind="ExternalInput")
    wkv = nc.dram_tensor("wkv", [hid, 128], BF16, kind="ExternalInput")
    wo = nc.dram_tensor("wo", [HD, hid], BF16, kind="ExternalInput")
    cos2 = nc.dram_tensor("cos2", [128, s_sz], F32, kind="ExternalInput")
    sinx = nc.dram_tensor("sinx", [128, s_sz], F32, kind="ExternalInput")
    maskd = nc.dram_tensor("maskd", [128, 128], F32, kind="ExternalInput")
    out = nc.dram_tensor("out", [b_sz * s_sz, hid], F32, kind="ExternalOutput")

    hsT_v = hsT.rearrange("(co p) n -> p co n", p=128)
    wq_v = wq.rearrange("(co p) m -> p co m", p=128)
    wkv_v = wkv.rearrange("(co p) m -> p co m", p=128)
    wo_v = wo.rearrange("(j p) n -> p j n", p=128)

    with TileContext(nc) as tc:
        with (
            tc.tile_pool(name="const", bufs=1) as cpool,
            tc.tile_pool(name="hst", bufs=2) as hpool,
            tc.tile_pool(name="perb", bufs=1) as bpool,
            tc.tile_pool(name="pt", bufs=2) as ptpool,
            tc.tile_pool(name="work", bufs=3) as wpool,
            tc.tile_pool(name="outsb", bufs=2) as opool,
            tc.tile_pool(name="psum", bufs=3, space="PSUM") as mmpool,
            tc.tile_pool(name="psum_pv", bufs=2, space="PSUM") as pvpool,
            tc.tile_pool(name="psum_tp", bufs=2, space="PSUM") as tppool,
        ):
            # ---- constants ----
            wq_t = cpool.tile([128, C, HQ * D], BF16, tag="wq")
            nc.sync.dma_start(wq_t[:], wq_v[:])
            wkv_t = cpool.tile([128, C, 128], BF16, tag="wkv")
            nc.sync.dma_start(wkv_t[:], wkv_v[:])
            wo_t = cpool.tile([128, HD // 128, hid], BF16, tag="wo")
            nc.sync.dma_start(wo_t[:], wo_v[:])
            cos_t = cpool.tile([128, s_sz], F32, tag="cos")
            nc.sync.dma_start(cos_t[:], cos2[:])
            sin_t = cpool.tile([128, s_sz], F32, tag="sin")
            nc.sync.dma_start(sin_t[:], sinx[:])
            mask_t = cpool.tile([128, 128], F32, tag="mask")
            nc.sync.dma_start(mask_t[:], maskd[:])
            ident = cpool.tile([128, 128], BF16, tag="ident")
            make_identity(nc, ident[:])

            def rope(dst, psum, s0, w, rows):
                """dst[bf16 SBUF [rows,w]] = RoPE(psum[:rows,:w]) using tables at cols s0:s0+w.

                rows is 64 (K) or 128 (2 stacked q-heads); per-32-row blocks:
                out = psum*cos + shift32(psum)*sinx, sinx sign-folded.
                """
                u = wpool.tile([128, sup], F32, tag="rope_u")
                t = wpool.tile([128, sup], F32, tag="rope_t")
                nc.vector.tensor_tensor(
                    u[:rows, :w], psum[:rows, :w], cos_t[:rows, s0:s0 + w],
                    mybir.AluOpType.mult)
                for o in range(0, rows, 64):
                    nc.vector.tensor_tensor(
                        t[o:o + 32, :w], psum[o + 32:o + 64, :w],
                        sin_t[o:o + 32, s0:s0 + w], mybir.AluOpType.mult)
                    nc.vector.tensor_tensor(
                        t[o + 32:o + 64, :w], psum[o:o + 32, :w],
                        sin_t[o + 32:o + 64, s0:s0 + w], mybir.AluOpType.mult)
                nc.vector.tensor_tensor(
                    dst, u[:rows, :w], t[:rows, :w], mybir.AluOpType.add)

            for b in range(b_sz):
                qt_b = bpool.tile([128, HQ // 2, s_sz], BF16, tag="qt")
                # K^T duplicated in both partition halves so QK matmuls can
                # match any q-head's base partition (and later row-pack).
                kt_b = bpool.tile([128, s_sz], BF16, tag="kt")
                vt_b = bpool.tile([64, s_sz], BF16, tag="vt")
                v_b = bpool.tile([128, n_st, 72], BF16, tag="v")
                attn_b = bpool.tile([128, n_st, HD], BF16, tag="attn")
                attnT_b = bpool.tile([128, HD // 128, s_sz], BF16, tag="attnT")
                nc.vector.memset(v_b[:, :, 64:65], 1.0)

                # ---- projections (stream hidden^T supertiles) ----
                for ss in range(s_sz // sup):
                    s0 = ss * sup
                    hst = hpool.tile([128, C, sup], BF16, tag="hst")
                    nc.sync.dma_start(
                        hst[:], hsT_v[:, :, b * s_sz + s0: b * s_sz + s0 + sup])
                    for hp in range(HQ // 2):
                        ps = mmpool.tile([128, sup], F32, tag="mm512")
                        for cc in range(C):
                            nc.tensor.matmul(
                                ps[:], wq_t[:, cc, hp * 128:(hp + 1) * 128],
                                hst[:, cc, :], start=(cc == 0), stop=(cc == C - 1))
                        rope(qt_b[:, hp, s0:s0 + sup], ps, s0, sup, 128)
                    ps = mmpool.tile([128, sup], F32, tag="mm512")
                    for cc in range(C):
                        nc.tensor.matmul(
                            ps[:], wkv_t[:, cc, :], hst[:, cc, :],
                            start=(cc == 0), stop=(cc == C - 1))
                    rope(kt_b[:64, s0:s0 + sup], ps, s0, sup, 64)
                    nc.vector.tensor_copy(
                        kt_b[64:128, s0:s0 + sup], kt_b[:64, s0:s0 + sup])
                    nc.vector.tensor_copy(vt_b[:, s0:s0 + sup], ps[64:128, :])

                # ---- V^T -> V tiles (PE transpose) ----
                for st in range(n_st):
                    pst = tppool.tile([128, 128], BF16, tag="tp")
                    nc.tensor.transpose(
                        pst[:, :64], vt_b[:, st * 128:(st + 1) * 128],
                        ident[:64, :64])
                    nc.vector.tensor_copy(v_b[:, st, :64], pst[:, :64])

                # ---- attention per local head ----
                for h in range(HQ):
                    o = (h % 2) * 64
                    qh = qt_b[o:o + 64, h // 2, :]
                    kth = kt_b[o:o + 64, :]
                    for qs in range(n_sup):
                        q0 = qs * sup
                        ptile = ptpool.tile([128, n_st, sup], BF16, tag="pt")
                        for kt in range(qs * n_qt + n_qt):
                            k0 = kt * 128
                            pss = mmpool.tile([128, sup], F32, tag="mm512")
                            if k0 < q0:
                                nc.tensor.matmul(
                                    pss[:], kth[:, k0:k0 + 128],
                                    qh[:, q0:q0 + sup], start=True, stop=True)
                                nc.scalar.activation(
                                    ptile[:, kt, :], pss[:],
                                    mybir.ActivationFunctionType.Exp, scale=SCALE)
                            else:
                                dq = k0 - q0
                                w = sup - dq
                                nc.tensor.matmul(
                                    pss[:, :128], kth[:, k0:k0 + 128],
                                    qh[:, k0:k0 + 128], start=True, stop=True)
                                if w > 128:
                                    nc.tensor.matmul(
                                        pss[:, 128:w], kth[:, k0:k0 + 128],
                                        qh[:, k0 + 128:q0 + sup],
                                        start=True, stop=True)
                                nc.vector.tensor_tensor(
                                    pss[:, :128], pss[:, :128], mask_t[:],
                                    mybir.AluOpType.add)
                                nc.scalar.activation(
                                    ptile[:, kt, dq:], pss[:, :w],
                                    mybir.ActivationFunctionType.Exp, scale=SCALE)
                        for qt4 in range(n_qt):
                            qt = qs * n_qt + qt4
                            psv = pvpool.tile([128, 72], F32, tag="pv")
                            for kt in range(qt + 1):
                                nc.tensor.matmul(
                                    psv[:, :65],
                                    ptile[:, kt, qt4 * 128:(qt4 + 1) * 128],
                                    v_b[:, kt, :65],
                                    start=(kt == 0), stop=(kt == qt))
                            rs = wpool.tile([128, 1], F32, tag="rs")
                            nc.vector.reciprocal(rs[:], psv[:, 64:65])
                            nc.vector.tensor_scalar_mul(
                                attn_b[:, qt, h * 64:(h + 1) * 64],
                                psv[:, :64], rs[:])

                # ---- attn -> attn^T (PE transpose of head pairs) ----
                for qt in range(n_st):
                    for j in range(HD // 128):
                        pst = tppool.tile([128, 128], BF16, tag="tp")
                        nc.tensor.transpose(
                            pst[:], attn_b[:, qt, j * 128:(j + 1) * 128],
                            ident[:])
                        nc.vector.tensor_copy(
                            attnT_b[:, j, qt * 128:(qt + 1) * 128], pst[:])

                # ---- output projection ----
                for st in range(n_st):
                    osb = opool.tile([128, hid], F32, tag="osb")
                    for cs in range(hid // 512):
                        pso = mmpool.tile([128, sup], F32, tag="mm512")
                        for j in range(HD // 128):
                            nc.tensor.matmul(
                                pso[:, :512],
                                attnT_b[:, j, st * 128:(st + 1) * 128],
                                wo_t[:, j, cs * 512:(cs + 1) * 512],
                                start=(j == 0), stop=(j == HD // 128 - 1))
                        nc.scalar.copy(osb[:, cs * 512:(cs + 1) * 512], pso[:, :512])
                    nc.sync.dma_start(
                        out[b * s_sz + st * 128: b * s_sz + (st + 1) * 128, :],
                        osb[:])
    nc.compile()
    return nc


def _rope_tables_np(seq_len, dim, base=10000.0):
    inv_freq = 1.0 / (base ** (np.arange(0, dim, 2, dtype=np.float32) / dim))
    t = np.arange(seq_len, dtype=np.float32)
    freqs = np.outer(t, inv_freq)
    emb = np.concatenate([freqs, freqs], axis=-1)
    return np.cos(emb), np.sin(emb)


def host_prep(hidden_states, cos, sin, Wq, Wk, Wv, Wo, s_sz=None, hid=None):
    """Slice/transposes/casts -> per-core input maps."""
    b_sz = hidden_states.shape[0]
    s_sz = s_sz or hidden_states.shape[1]
    hid = hid or hidden_states.shape[2]
    bf = ml_dtypes.bfloat16

    hsT = np.ascontiguousarray(
        hidden_states.reshape(b_sz * s_sz, hid).T).astype(bf)

    cosT = np.asarray(cos, np.float32).T          # [64, S]
    sinT = np.asarray(sin, np.float32).T
    cos2 = np.concatenate([cosT, cosT], axis=0)   # [128, S]
    sinx = np.concatenate(
        [-sinT[:32], sinT[32:64], -sinT[:32], sinT[32:64]], axis=0)
    cos2 = np.ascontiguousarray(cos2, dtype=np.float32)
    sinx = np.ascontiguousarray(sinx, dtype=np.float32)

    kk, qq = np.meshgrid(np.arange(128), np.arange(128), indexing="ij")
    maskd = np.where(kk <= qq, 0.0, NEG).astype(np.float32)

    in_maps = []
    for c in range(NCORES):
        wq_c = np.ascontiguousarray(Wq[:, c * HD:(c + 1) * HD]).astype(bf)
        wkv_c = np.concatenate(
            [Wk[:, c * D:(c + 1) * D], Wv[:, c * D:(c + 1) * D]], axis=1
        ).astype(bf)
        wo_c = np.ascontiguousarray(Wo[c * HD:(c + 1) * HD, :]).astype(bf)
        in_maps.append({
            "hsT": hsT, "wq": wq_c, "wkv": np.ascontiguousarray(wkv_c),
            "wo": wo_c, "cos2": cos2, "sinx": sinx, "maskd": maskd,
        })
    return in_maps


def kernel_run(hidden_states, cos, sin, attention_mask, Wq, Wk, Wv, Wo,
               **spmd_kwargs):
    from concourse.bass_utils import run_bass_kernel_spmd

    hidden_states = np.asarray(hidden_states, np.float32)
    in_maps = host_prep(hidden_states, cos, sin,
                        np.asarray(Wq, np.float32), np.asarray(Wk, np.float32),
                        np.asarray(Wv, np.float32), np.asarray(Wo, np.float32))
    nc = build_nc()
    res = run_bass_kernel_spmd(nc, in_maps, core_ids=list(range(NCORES)),
                               **spmd_kwargs)
    acc = np.zeros((B * S, HID), np.float64)
    for r in res.results:
        acc += r["out"].astype(np.float64)
    return acc.reshape(B, S, HID).astype(np.float32), res


def kernel(hidden_states, cos, sin, attention_mask, Wq, Wk, Wv, Wo):
    out, _ = kernel_run(hidden_states, cos, sin, attention_mask,
                        Wq, Wk, Wv, Wo)
    return out


if __name__ == "__main__":
    pass
